# revision 17
# baseline (speedup 1.0000x reference)
"""Balanced BCE loss on 8 Trainium2 NeuronCores.

loss = -sum_i [ beta_i * sum_j(t_ij * ln(p_ij))
                + (1-beta_i) * sum_j((1-t_ij) * ln(1-p_ij)) ]
beta_i = 1 - mean_j(t_ij)

Host casts inputs to bf16 (halves HBM traffic; quantization error on the
summed loss is ~1e-4 relative) and reshapes each core's 8 rows to a flat
[128, 16384] layout where row r owns partitions 16r..16r+15.

Three row-reductions only:
  S = sum_j(t)   A = sum_j(t * ln p)   C' = sum_j((1-t) * ln(1-p))
  beta_i = 1 - S_i/N ;  loss = -sum_rows[ beta*A + (1-beta)*C' ]
The S reduction consumes raw t so it closes as soon as the last t
chunk lands, letting the S+A reduce overlap the final C matmul.

Engine assignment per column chunk [128, F] (graduated grid: big chunks
mid-stream for low instruction overhead, small last chunk for a short
tail):
  - ACT: lnp = Ln(p) bf16; ln1mp = Ln(1-p) bf16.  ACT is the bottleneck:
         two passes over every element = (2*16384 + ovh)/1.2GHz ~ 30us.
  - DVE: u = 1-t (tensor_scalar, 4x); m1 = t*lnp, m2 = u*ln1mp (TT, 2x).
  - PE: selector-matrix matmuls W[128,8]^T @ {u, m1, m2} accumulate
        S'/A/C' for all 8 rows at once into one fused [8, 3*256] PSUM
        tile (per-chunk S|A|C blocks so a stalled C block never delays
        S/A; LDWEIGHTS hides under the previous matmul's drain).
  - Tail: one fused 3-region DVE tensor_reduce + one 96B output DMA.
"""

from contextlib import ExitStack

import numpy as np
import ml_dtypes

import concourse.bass as bass
import concourse.mybir as mybir
import concourse.tile as tile
from concourse import bacc
from concourse.bass_utils import run_bass_kernel_spmd

B, N = 64, 262144
NCORES = 8
ROWS = B // NCORES  # rows per core
P = 128  # SBUF partitions
NF = ROWS * N // P  # 16384 free-dim cols per partition
PPR = P // ROWS  # 16 partitions per row

AF = mybir.ActivationFunctionType
ALU = mybir.AluOpType
AX = mybir.AxisListType
f32 = mybir.dt.float32
bf16 = mybir.dt.bfloat16
np_bf16 = ml_dtypes.bfloat16

CH = 256  # PSUM accumulator width / matmul moving window
CHUNKS = [1024, 4096, 5632, 3584, 1536, 512]
assert sum(CHUNKS) == NF and all(c % CH == 0 for c in CHUNKS)

# test.py can flip this to capture an NTFF profile of the run
TRACE = False
LAST = None  # BassKernelResults of the most recent kernel() call


def _emit(tc, out_ap, inp_ap, tgt_ap, wbf_ap):
    nc = tc.nc
    nch = len(CHUNKS)
    offs = [sum(CHUNKS[:i]) for i in range(nch)]

    with ExitStack() as ctx:
        singles = ctx.enter_context(tc.tile_pool(name="s", bufs=1))
        psum_pool = ctx.enter_context(tc.tile_pool(name="ps", bufs=1, space="PSUM"))

        wbf = singles.tile([P, ROWS], bf16, tag="wbf")
        stats = singles.tile([ROWS, 3], f32, tag="stats")

        psall = psum_pool.tile([ROWS, 3 * CH], f32, tag="psall", name="psall")
        psS = psall[:, 0 * CH : 1 * CH]
        psA = psall[:, 1 * CH : 2 * CH]
        psC = psall[:, 2 * CH : 3 * CH]

        # stage all loads upfront on the SP queue; p chunks run one ahead
        # of t (ACT's critical path), selector W after p1 (PE needs it
        # only once t0's first products exist)
        ptiles = [singles.tile([P, F], bf16, tag=f"p{c}", name=f"p{c}") for c, F in enumerate(CHUNKS)]
        ttiles = [singles.tile([P, F], bf16, tag=f"t{c}", name=f"t{c}") for c, F in enumerate(CHUNKS)]
        order = [("p", 0), ("p", 1), ("w", 0), ("t", 0), ("p", 2), ("t", 1),
                 ("p", 3), ("t", 2), ("p", 4), ("t", 3), ("p", 5), ("t", 4),
                 ("t", 5)]
        for kind, c in order:
            if kind == "w":
                nc.sync.dma_start(wbf[:], wbf_ap)
                continue
            src = inp_ap if kind == "p" else tgt_ap
            dst = ptiles[c] if kind == "p" else ttiles[c]
            nc.sync.dma_start(dst[:], src[:, offs[c] : offs[c] + CHUNKS[c]])

        for c, F in enumerate(CHUNKS):
            p_t = ptiles[c][:]
            t_t = ttiles[c][:]

            lnp = singles.tile([P, F], bf16, tag=f"lnp{c}", name=f"lnp{c}")
            nc.scalar.activation(lnp[:], p_t, AF.Ln)
            l1mp = singles.tile([P, F], bf16, tag=f"l1mp{c}", name=f"l1mp{c}")
            nc.scalar.activation(l1mp[:], p_t, AF.Ln, scale=-1.0, bias=1.0)

            # u = 1-t in one 4x tensor_scalar pass (STT would be 1x);
            # products overwrite the logs in place (their only consumer;
            # same-index elementwise on DVE is stream-safe)
            u = singles.tile([P, F], bf16, tag=f"u{c}", name=f"u{c}")
            nc.vector.tensor_scalar(u[:], t_t, -1.0, 1.0, ALU.mult, ALU.add)
            m1 = lnp
            nc.vector.tensor_mul(m1[:], t_t, lnp[:])
            m2 = l1mp
            nc.vector.tensor_mul(m2[:], u[:], l1mp[:])

            # S-set consumes raw t, so psS closes as soon as t5 lands
            for ps, src_t in ((psS, t_t), (psA, m1), (psC, m2)):
                for j in range(F // CH):
                    sl = slice(j * CH, (j + 1) * CH)
                    nc.tensor.matmul(
                        ps, wbf[:], src_t[:, sl],
                        start=(c == 0 and j == 0),
                        stop=(c == nch - 1 and j == F // CH - 1),
                        skip_group_check=True,
                    )

        # two-stage final reduce: S+A regions close before the last C
        # matmul, so their reduce overlaps it
        nc.vector.tensor_reduce(
            stats[:, 0:2], psall[:, 0 : 2 * CH].rearrange("p (s c) -> p s c", s=2),
            axis=AX.X, op=ALU.add,
        )
        nc.vector.tensor_reduce(stats[:, 2:3], psC, axis=AX.X, op=ALU.add)
        nc.sync.dma_start(out_ap, stats[:])


_PROG_CACHE = {}


def _build_program():
    key = "v10"
    if key not in _PROG_CACHE:
        nc = bacc.Bacc("TRN2", target_bir_lowering=False, debug=False)
        inp = nc.dram_tensor("input", [P, NF], bf16, kind="ExternalInput").ap()
        tgt = nc.dram_tensor("target", [P, NF], bf16, kind="ExternalInput").ap()
        wbf_d = nc.dram_tensor("wsel_bf", [P, ROWS], bf16, kind="ExternalInput").ap()
        out = nc.dram_tensor("partials", [ROWS, 3], f32, kind="ExternalOutput").ap()
        with tile.TileContext(nc) as tc:
            _emit(tc, out, inp, tgt, wbf_d)
        nc.finalize()
        _PROG_CACHE[key] = nc
    return _PROG_CACHE[key]


def kernel(input, target):
    global LAST
    input = np.asarray(input)
    target = np.asarray(target)
    assert input.shape == (B, N) and target.shape == (B, N)

    inp_bf = np.ascontiguousarray(input).astype(np_bf16)
    tgt_bf = np.ascontiguousarray(target).astype(np_bf16)

    nc = _build_program()
    wsel = np.zeros((P, ROWS), dtype=np.float32)
    for r in range(ROWS):
        wsel[r * PPR : (r + 1) * PPR, r] = 1.0
    wsel_bf = wsel.astype(np_bf16)
    in_maps = [
        {
            "input": inp_bf[c * ROWS : (c + 1) * ROWS].reshape(P, NF),
            "target": tgt_bf[c * ROWS : (c + 1) * ROWS].reshape(P, NF),
            "wsel_bf": wsel_bf,
        }
        for c in range(NCORES)
    ]
    res = run_bass_kernel_spmd(nc, in_maps, core_ids=list(range(NCORES)), trace=TRACE)
    LAST = res

    total = np.float64(0.0)
    for c in range(NCORES):
        part = res.results[c]["partials"].astype(np.float64)  # [ROWS, 3]
        S, A, Cp = part[:, 0], part[:, 1], part[:, 2]
        beta = 1.0 - S / N
        total += np.sum(beta * A + (1.0 - beta) * Cp)
    return np.float32(-total)


# revision 18
# speedup vs baseline: 1.0585x; 1.0585x over previous
"""Balanced BCE loss on 8 Trainium2 NeuronCores.

loss = -sum_i [ beta_i * sum_j(t_ij * ln(p_ij))
                + (1-beta_i) * sum_j((1-t_ij) * ln(1-p_ij)) ]
beta_i = 1 - mean_j(t_ij)

Host casts inputs to bf16 (halves HBM traffic; quantization error on the
summed loss is ~1e-4 relative) and reshapes each core's 8 rows to a flat
[128, 16384] layout where row r owns partitions 16r..16r+15.

Three row-reductions only:
  S = sum_j(t)   A = sum_j(t * ln p)   C' = sum_j((1-t) * ln(1-p))
  beta_i = 1 - S_i/N ;  loss = -sum_rows[ beta*A + (1-beta)*C' ]
The S reduction consumes raw t so it closes as soon as the last t
chunk lands, letting the S+A reduce overlap the final C matmul.

Engine assignment per column chunk [128, F] (graduated grid: big chunks
mid-stream for low instruction overhead, small last chunk for a short
tail):
  - ACT: lnp = Ln(p) bf16; ln1mp = Ln(1-p) bf16.  ACT is the bottleneck:
         two passes over every element = (2*16384 + ovh)/1.2GHz ~ 30us.
  - DVE: u = 1-t (tensor_scalar, 4x); m1 = t*lnp, m2 = u*ln1mp (TT, 2x).
  - PE: selector-matrix matmuls W[128,8]^T @ {u, m1, m2} accumulate
        S'/A/C' for all 8 rows at once into one fused [8, 3*256] PSUM
        tile (per-chunk S|A|C blocks so a stalled C block never delays
        S/A; LDWEIGHTS hides under the previous matmul's drain).
  - Tail: one fused 3-region DVE tensor_reduce + one 96B output DMA.
"""

from contextlib import ExitStack

import numpy as np
import ml_dtypes

import concourse.bass as bass
import concourse.mybir as mybir
import concourse.tile as tile
from concourse import bacc
from concourse.bass_utils import run_bass_kernel_spmd

B, N = 64, 262144
NCORES = 8
ROWS = B // NCORES  # rows per core
P = 128  # SBUF partitions
NF = ROWS * N // P  # 16384 free-dim cols per partition
PPR = P // ROWS  # 16 partitions per row

AF = mybir.ActivationFunctionType
ALU = mybir.AluOpType
AX = mybir.AxisListType
f32 = mybir.dt.float32
bf16 = mybir.dt.bfloat16
np_bf16 = ml_dtypes.bfloat16

CH = 256  # PSUM accumulator width / matmul moving window
CHUNKS = [1024, 3584, 6144, 3584, 1536, 512]
assert sum(CHUNKS) == NF and all(c % CH == 0 for c in CHUNKS)

# test.py can flip this to capture an NTFF profile of the run
TRACE = False
LAST = None  # BassKernelResults of the most recent kernel() call


def _emit(tc, out_ap, inp_ap, tgt_ap, wbf_ap):
    nc = tc.nc
    nch = len(CHUNKS)
    offs = [sum(CHUNKS[:i]) for i in range(nch)]

    with ExitStack() as ctx:
        singles = ctx.enter_context(tc.tile_pool(name="s", bufs=1))
        psum_pool = ctx.enter_context(tc.tile_pool(name="ps", bufs=1, space="PSUM"))

        wbf = singles.tile([P, ROWS], bf16, tag="wbf")
        stats = singles.tile([ROWS, 3], f32, tag="stats")

        psall = psum_pool.tile([ROWS, 3 * CH], f32, tag="psall", name="psall")
        psS = psall[:, 0 * CH : 1 * CH]
        psA = psall[:, 1 * CH : 2 * CH]
        psC = psall[:, 2 * CH : 3 * CH]

        # stage all loads upfront on the SP queue; p chunks run one ahead
        # of t (ACT's critical path), selector W after p1 (PE needs it
        # only once t0's first products exist)
        ptiles = [singles.tile([P, F], bf16, tag=f"p{c}", name=f"p{c}") for c, F in enumerate(CHUNKS)]
        ttiles = [singles.tile([P, F], bf16, tag=f"t{c}", name=f"t{c}") for c, F in enumerate(CHUNKS)]
        order = [("p", 0), ("p", 1), ("w", 0), ("t", 0), ("p", 2), ("t", 1),
                 ("p", 3), ("t", 2), ("p", 4), ("t", 3), ("p", 5), ("t", 4),
                 ("t", 5)]
        for kind, c in order:
            if kind == "w":
                nc.sync.dma_start(wbf[:], wbf_ap)
                continue
            src = inp_ap if kind == "p" else tgt_ap
            dst = ptiles[c] if kind == "p" else ttiles[c]
            nc.sync.dma_start(dst[:], src[:, offs[c] : offs[c] + CHUNKS[c]])

        for c, F in enumerate(CHUNKS):
            p_t = ptiles[c][:]
            t_t = ttiles[c][:]

            lnp = singles.tile([P, F], bf16, tag=f"lnp{c}", name=f"lnp{c}")
            nc.scalar.activation(lnp[:], p_t, AF.Ln)
            l1mp = singles.tile([P, F], bf16, tag=f"l1mp{c}", name=f"l1mp{c}")
            nc.scalar.activation(l1mp[:], p_t, AF.Ln, scale=-1.0, bias=1.0)

            # u = 1-t in one 4x tensor_scalar pass (STT would be 1x);
            # products overwrite the logs in place (their only consumer;
            # same-index elementwise on DVE is stream-safe)
            u = singles.tile([P, F], bf16, tag=f"u{c}", name=f"u{c}")
            nc.vector.tensor_scalar(u[:], t_t, -1.0, 1.0, ALU.mult, ALU.add)
            m1 = lnp
            nc.vector.tensor_mul(m1[:], t_t, lnp[:])
            m2 = l1mp
            nc.vector.tensor_mul(m2[:], u[:], l1mp[:])

            # S-set consumes raw t, so psS closes as soon as t5 lands
            for ps, src_t in ((psS, t_t), (psA, m1), (psC, m2)):
                for j in range(F // CH):
                    sl = slice(j * CH, (j + 1) * CH)
                    nc.tensor.matmul(
                        ps, wbf[:], src_t[:, sl],
                        start=(c == 0 and j == 0),
                        stop=(c == nch - 1 and j == F // CH - 1),
                        skip_group_check=True,
                    )

        # two-stage final reduce: S+A regions close before the last C
        # matmul, so their reduce overlaps it
        nc.vector.tensor_reduce(
            stats[:, 0:2], psall[:, 0 : 2 * CH].rearrange("p (s c) -> p s c", s=2),
            axis=AX.X, op=ALU.add,
        )
        nc.vector.tensor_reduce(stats[:, 2:3], psC, axis=AX.X, op=ALU.add)
        nc.sync.dma_start(out_ap, stats[:])


_PROG_CACHE = {}


def _build_program():
    key = "v11"
    if key not in _PROG_CACHE:
        nc = bacc.Bacc("TRN2", target_bir_lowering=False, debug=False)
        inp = nc.dram_tensor("input", [P, NF], bf16, kind="ExternalInput").ap()
        tgt = nc.dram_tensor("target", [P, NF], bf16, kind="ExternalInput").ap()
        wbf_d = nc.dram_tensor("wsel_bf", [P, ROWS], bf16, kind="ExternalInput").ap()
        out = nc.dram_tensor("partials", [ROWS, 3], f32, kind="ExternalOutput").ap()
        with tile.TileContext(nc) as tc:
            _emit(tc, out, inp, tgt, wbf_d)
        nc.finalize()
        _PROG_CACHE[key] = nc
    return _PROG_CACHE[key]


def kernel(input, target):
    global LAST
    input = np.asarray(input)
    target = np.asarray(target)
    assert input.shape == (B, N) and target.shape == (B, N)

    inp_bf = np.ascontiguousarray(input).astype(np_bf16)
    tgt_bf = np.ascontiguousarray(target).astype(np_bf16)

    nc = _build_program()
    wsel = np.zeros((P, ROWS), dtype=np.float32)
    for r in range(ROWS):
        wsel[r * PPR : (r + 1) * PPR, r] = 1.0
    wsel_bf = wsel.astype(np_bf16)
    in_maps = [
        {
            "input": inp_bf[c * ROWS : (c + 1) * ROWS].reshape(P, NF),
            "target": tgt_bf[c * ROWS : (c + 1) * ROWS].reshape(P, NF),
            "wsel_bf": wsel_bf,
        }
        for c in range(NCORES)
    ]
    res = run_bass_kernel_spmd(nc, in_maps, core_ids=list(range(NCORES)), trace=TRACE)
    LAST = res

    total = np.float64(0.0)
    for c in range(NCORES):
        part = res.results[c]["partials"].astype(np.float64)  # [ROWS, 3]
        S, A, Cp = part[:, 0], part[:, 1], part[:, 2]
        beta = 1.0 - S / N
        total += np.sum(beta * A + (1.0 - beta) * Cp)
    return np.float32(-total)
